# revision 9
# baseline (speedup 1.0000x reference)
"""Trainium2 Bass kernel for nn_LowLevelActionHeadLayer (8-core SPMD).

Data-parallel over batch: 16 batches -> 2 per NeuronCore. Per core, the full
layer runs fused on-chip:
  state-KV projection (1024x2048 @ 2048x512, bf16 PE),
  RMSNorm1 -> QKV -> RoPE -> block-sparse attention (S^T layout, ones-column
  softmax denominators) -> sigmoid gate -> out-proj -> residual ->
  RMSNorm2 -> GELU FFN -> residual.
Activations cross the TensorEngine in bf16 (fp32 PSUM accumulation); softmax
and both residual trunks stay fp32. The big VLM activation transpose uses
f32->bf16 cast-DMA (SWDGE) + xbar block-transpose DMA (HWDGE).
"""

import sys

for _p in ('/opt/trn_rl_repo', '/opt/pypackages', '/root/.axon_site'):
    if _p not in sys.path:
        sys.path.insert(0, _p)

import numpy as np
import ml_dtypes

import concourse.bass as bass
import concourse.mybir as mybir
import concourse.tile as tile
import bass_rust

F32 = mybir.dt.float32
BF16 = mybir.dt.bfloat16
AF = mybir.ActivationFunctionType
ALU = mybir.AluOpType
BF16NP = ml_dtypes.bfloat16

CORES = 8
B = 16
BL = B // CORES          # batches per core
NK = 512                 # state tokens per vlm side
P, PR, M = 256, 32, 64
LQ = P + 1 + PR + M      # 353
LK = NK + LQ             # 865
D = 512
H = 8
DH = 64
EMB = 2048
SCALE = DH ** -0.5
EPS = 1e-6

IT_CNT = [128, 128, LQ - 256]        # query-token tiles (128/128/97)
JT_CNT = [128, 128, 128, 128, 128, 128, LK - 768]  # key-token tiles


# ---------------------------------------------------------------- host tables
def _rope_tables(positions, scale):
    """cos/sin tables transposed to [DH, n], sin sign-folded for the
    half-split rotation, tiled to 128 partitions (2 heads per tile row
    group)."""
    inv = 1.0 / (10000.0 ** (np.arange(0, DH, 2, dtype=np.float64) / DH))
    freqs = np.outer(np.asarray(positions, np.float64), inv)      # [n, 32]
    cos = np.concatenate([np.cos(freqs), np.cos(freqs)], -1).T    # [64, n]
    sin = np.concatenate([np.sin(freqs), np.sin(freqs)], -1).T
    sin_signed = sin.copy()
    sin_signed[:DH // 2] *= -1.0
    cos = np.tile(cos * scale, (2, 1)).astype(np.float32)         # [128, n]
    sin_signed = np.tile(sin_signed * scale, (2, 1)).astype(np.float32)
    return np.ascontiguousarray(cos), np.ascontiguousarray(sin_signed)


def _mask6_table():
    """Additive mask for key tile jt=6 (j = 768..864) in S^T layout
    [j-row, i-col]; rows 97..127 unused."""
    mk = np.zeros((128, LQ), np.float32)
    limit = np.full(LQ, LK, np.int64)
    limit[:P] = P + NK                      # planning rows: j < 768
    limit[P:P + 1 + PR] = P + NK + 1 + PR   # t+proprio rows: j < 801
    j = 768 + np.arange(97)
    mk[:97] = np.where(j[:, None] >= limit[None, :], -30000.0, 0.0)
    return mk


# ---------------------------------------------------- BIR post-pass (walrus)
def _split_oversized_waits(nc, ctrl_max=1, other_max=1):
    """This walrus build caps per-instruction sync-wait commands (CTRL-class
    ops take only one). Hoist excess waits onto preceding single-wait nops on
    the same engine; in-order issue preserves semantics."""
    ctrl_types = ('InstDrain', 'InstNoOp')
    ctr = 0
    for f in nc.m.functions:
        for bb in f.blocks:
            out, changed = [], False
            for inst in bb.instructions:
                si = inst.sync_info
                if si is not None:
                    cap = ctrl_max if type(inst).__name__ in ctrl_types \
                        else other_max
                    if len(si.on_wait) > cap:
                        waits = list(si.on_wait)
                        keep, rest = waits[-cap:], waits[:-cap]
                        while rest:
                            chunk, rest = rest[:ctrl_max], rest[ctrl_max:]
                            nop = mybir.InstNoOp(name=f"wsplit-{ctr}")
                            ctr += 1
                            nop.engine = inst.engine
                            nop.sync_info = bass_rust.SyncInfo(
                                on_wait=chunk, on_update=[])
                            out.append(nop)
                        inst.sync_info = bass_rust.SyncInfo(
                            on_wait=keep, on_update=list(si.on_update))
                        changed = True
                out.append(inst)
            if changed:
                bb.instructions = out


# ------------------------------------------------------------------ builder
def build_nc():
    nc = bass.Bass()

    d_vk = nc.declare_dram_parameter("vk", [BL, NK, EMB], F32, isOutput=False)
    d_vv = nc.declare_dram_parameter("vv", [BL, NK, EMB], F32, isOutput=False)
    d_or = nc.declare_dram_parameter("orig", [BL, LQ, D], F32, isOutput=False)
    d_wskv = nc.declare_dram_parameter("wskv", [EMB, D], BF16, isOutput=False)
    d_wqk = nc.declare_dram_parameter("wqk", [D, 2 * D], BF16, isOutput=False)
    d_wv = nc.declare_dram_parameter("wv", [D, D], BF16, isOutput=False)
    d_wg = nc.declare_dram_parameter("wg", [D, D], BF16, isOutput=False)
    d_wo = nc.declare_dram_parameter("wo", [D, D], BF16, isOutput=False)
    d_w1 = nc.declare_dram_parameter("w1", [D, D], BF16, isOutput=False)
    d_w2 = nc.declare_dram_parameter("w2", [D, D], BF16, isOutput=False)
    d_cq = nc.declare_dram_parameter("cq", [128, LQ], F32, isOutput=False)
    d_sq = nc.declare_dram_parameter("sq", [128, LQ], F32, isOutput=False)
    d_cks = nc.declare_dram_parameter("cks", [128, LQ], F32, isOutput=False)
    d_sks = nc.declare_dram_parameter("sks", [128, LQ], F32, isOutput=False)
    d_ckst = nc.declare_dram_parameter("ckst", [128, NK], F32, isOutput=False)
    d_skst = nc.declare_dram_parameter("skst", [128, NK], F32, isOutput=False)
    d_mk6 = nc.declare_dram_parameter("mk6", [128, LQ], F32, isOutput=False)
    d_out = nc.declare_dram_parameter("out", [BL, LQ, D], F32, isOutput=True)

    with tile.TileContext(nc) as tc:
        _body(nc, tc, locals())
    return nc


def _body(nc, tc, d):
    from contextlib import ExitStack
    ctx = ExitStack()
    pool = {}
    for name, bufs, space in [
        ("consts", 1, "SBUF"), ("xt", 1, "SBUF"), ("stage", 2, "SBUF"),
        ("kst", 5, "SBUF"), ("vsb", 14, "SBUF"), ("onat", 6, "SBUF"),
        ("rtmp", 1, "SBUF"), ("sq", 1, "SBUF"), ("xT", 2, "SBUF"),
        ("qkT", 8, "SBUF"), ("pt", 10, "SBUF"), ("rec", 2, "SBUF"),
        ("bc", 2, "SBUF"), ("ao", 8, "SBUF"), ("sg", 4, "SBUF"),
        ("ybf", 2, "SBUF"), ("ynat", 2, "SBUF"), ("stats", 4, "SBUF"),
        ("n2bf", 3, "SBUF"), ("n2T", 2, "SBUF"), ("h1T", 8, "SBUF"),
        ("f2bf", 2, "SBUF"), ("f2nat", 2, "SBUF"),
        ("ps_mm", 2, "PSUM"), ("ps_s", 3, "PSUM"), ("ps_o", 2, "PSUM"),
        ("ps_b", 1, "PSUM"),
    ]:
        pool[name] = ctx.enter_context(
            tc.tile_pool(name=name, bufs=bufs, space=space))
    c = pool["consts"]

    # ---- constants into SBUF
    wskv = [c.tile([128, D], BF16, tag=f"wskv{i}", name=f"wskv{i}") for i in range(16)]
    for i in range(16):
        nc.sync.dma_start(out=wskv[i], in_=d["d_wskv"][i * 128:(i + 1) * 128, :])
    wqk = [c.tile([128, 2 * D], BF16, tag=f"wqk{i}", name=f"wqk{i}") for i in range(4)]
    wv, wg, wo, w1, w2 = ([c.tile([128, D], BF16, tag=f"w{n}{i}", name=f"w{n}{i}")
                           for i in range(4)]
                          for n in ("v", "g", "o", "1", "2"))
    for i in range(4):
        nc.sync.dma_start(out=wqk[i], in_=d["d_wqk"][i * 128:(i + 1) * 128, :])
        for t, dd in ((wv, "d_wv"), (wg, "d_wg"), (wo, "d_wo"),
                      (w1, "d_w1"), (w2, "d_w2")):
            nc.sync.dma_start(out=t[i], in_=d[dd][i * 128:(i + 1) * 128, :])
    tabs = {}
    for nm in ("cq", "sq", "cks", "sks", "ckst", "skst", "mk6"):
        shp = [128, NK] if nm in ("ckst", "skst") else [128, LQ]
        tabs[nm] = c.tile(shp, F32, tag=nm, name=nm)
        nc.sync.dma_start(out=tabs[nm], in_=d["d_" + nm][:, :])
    ones64 = c.tile([1, 64], F32, tag="ones64", name="ones64")
    nc.vector.memset(ones64, 1.0)
    epsb = c.tile([128, 1], F32, tag="epsb", name="epsb")
    nc.vector.memset(epsb, EPS)

    def rope(ps, cosT, sinT, out_bf, n):
        """out_bf[128, n] (bf16) = rope(ps[128, n]) with tables [128, >=n]."""
        t1 = pool["rtmp"].tile([128, NK], F32, tag="t1", name="t1")
        t2 = pool["rtmp"].tile([128, NK], F32, tag="t2", name="t2")
        nc.vector.tensor_tensor(out=t1[:, :n], in0=ps, in1=cosT[:, :n],
                                op=ALU.mult)
        for g in (0, 64):
            nc.vector.tensor_tensor(out=t2[g:g + 32, :n],
                                    in0=ps[g + 32:g + 64, :],
                                    in1=sinT[g:g + 32, :n], op=ALU.mult)
            nc.vector.tensor_tensor(out=t2[g + 32:g + 64, :n],
                                    in0=ps[g:g + 32, :],
                                    in1=sinT[g + 32:g + 64, :n], op=ALU.mult)
        nc.vector.tensor_tensor(out=out_bf, in0=t1[:, :n], in1=t2[:, :n],
                                op=ALU.add)

    # per-b state
    kst = [[None] * 4 for _ in range(BL)]   # roped state keys^T, bf16
    vsb = [[None] * 7 for _ in range(BL)]   # [128, 8, 65] values + ones col
    qkT = [[None] * 8 for _ in range(BL)]   # roped q^T (0-3) / self k^T (4-7)
    onat = [[None] * 3 for _ in range(BL)]  # orig -> r1 -> r2 (fp32, in-place)
    xT = [None] * BL                        # [128, 4, 384] normed x^T bf16
    ao = [[None] * 4 for _ in range(BL)]    # allout^T (normalized, gated)

    def vlm_phase(b):
        xt = pool["xt"].tile([128, 16, 2 * NK], BF16, tag="xt", name="xt")
        for side, src in ((0, d["d_vk"]), (1, d["d_vv"])):
            for tt in range(4):
                st = pool["stage"].tile([128, EMB], BF16, tag="stage", name="stage")
                nc.gpsimd.dma_start(
                    out=st, in_=src[b, tt * 128:(tt + 1) * 128, :])
                nc.sync.dma_start(
                    out=xt[:, :, side * NK + tt * 128: side * NK + (tt + 1) * 128],
                    in_=st.rearrange("p (c q) -> p c q", c=16),
                    transpose=True)
        return xt

    def state_phase(b, xt):
        # keys^T: [dout-tile, 512 state tokens], then RoPE
        for dt in range(4):
            ps = pool["ps_mm"].tile([128, NK], F32, tag="mm", name="mm")
            for ec in range(16):
                nc.tensor.matmul(ps, wskv[ec][:, dt * 128:(dt + 1) * 128],
                                 xt[:, ec, 0:NK], start=(ec == 0),
                                 stop=(ec == 15))
            kst[b][dt] = pool["kst"].tile([128, NK], BF16, tag="kst", name="kst")
            rope(ps, tabs["ckst"], tabs["skst"], kst[b][dt], NK)
        # values natural -> strided head layout with ones column
        for tt in range(4):
            ps = pool["ps_mm"].tile([128, NK], F32, tag="mm", name="mm")
            for ec in range(16):
                nc.tensor.matmul(ps, xt[:, ec, NK + tt * 128:NK + (tt + 1) * 128],
                                 wskv[ec][:, :], start=(ec == 0),
                                 stop=(ec == 15))
            vt = pool["vsb"].tile([128, H, DH + 1], BF16, tag="vsb", name="vsb")
            nc.gpsimd.memset(vt[:, :, DH:DH + 1], 1.0)
            nc.vector.tensor_copy(vt[:, :, 0:DH],
                                  ps.rearrange("p (h e) -> p h e", h=H))
            vsb[b][tt] = vt

    def norm1_phase(b):
        xTb = pool["xT"].tile([128, 4, 384], BF16, tag="xT", name="xT")
        for it in range(3):
            cnt = IT_CNT[it]
            o = pool["onat"].tile([128, D], F32, tag="onat", name="onat")
            nc.sync.dma_start(out=o[0:cnt, :],
                              in_=d["d_or"][b, it * 128:it * 128 + cnt, :])
            onat[b][it] = o
            sqt = pool["sq"].tile([128, D], F32, tag="sq", name="sq")
            ss = pool["stats"].tile([128, 1], F32, tag="ss", name="ss")
            nc.scalar.activation(sqt[0:cnt, :], o[0:cnt, :], AF.Square,
                                 accum_out=ss[0:cnt, :])
            std = pool["stats"].tile([128, 1], F32, tag="std", name="std")
            nc.scalar.activation(std[0:cnt, :], ss[0:cnt, :], AF.Sqrt,
                                 scale=1.0 / D, bias=epsb[0:cnt, :])
            rstd = pool["stats"].tile([128, 1], F32, tag="rstd", name="rstd")
            nc.vector.reciprocal(rstd[0:cnt, :], std[0:cnt, :])
            xn = pool["n2bf"].tile([128, D], BF16, tag="xn", name="xn")
            if cnt < 128:
                nc.gpsimd.memset(xn[96:128, :], 0.0)
            nc.vector.tensor_scalar(out=xn[0:cnt, :], in0=o[0:cnt, :],
                                    scalar1=rstd[0:cnt, :], scalar2=None,
                                    op0=ALU.mult)
            nc.sync.dma_start(out=xTb[:, :, it * 128:(it + 1) * 128],
                              in_=xn.rearrange("p (c q) -> p c q", c=4),
                              transpose=True)
        xT[b] = xTb

    def qkv_phase(b):
        for ot in range(8):
            ps = pool["ps_mm"].tile([128, NK], F32, tag="mm", name="mm")
            for fc in range(4):
                nc.tensor.matmul(ps[:, 0:LQ],
                                 wqk[fc][:, ot * 128:(ot + 1) * 128],
                                 xT[b][:, fc, 0:LQ], start=(fc == 0),
                                 stop=(fc == 3))
            qt = pool["qkT"].tile([128, LQ], BF16, tag="qkT", name="qkT")
            if ot < 4:
                rope(ps[:, 0:LQ], tabs["cq"], tabs["sq"], qt, LQ)
            else:
                rope(ps[:, 0:LQ], tabs["cks"], tabs["sks"], qt, LQ)
            qkT[b][ot] = qt
        for it in range(3):
            cnt = IT_CNT[it]
            ps = pool["ps_mm"].tile([128, NK], F32, tag="mm", name="mm")
            for fc in range(4):
                nc.tensor.matmul(ps[0:cnt, :],
                                 xT[b][:, fc, it * 128:it * 128 + cnt],
                                 wv[fc][:, :], start=(fc == 0), stop=(fc == 3))
            vt = pool["vsb"].tile([128, H, DH + 1], BF16, tag="vsb", name="vsb")
            nc.gpsimd.memset(vt[0:cnt, :, DH:DH + 1], 1.0)
            nc.vector.tensor_copy(vt[0:cnt, :, 0:DH],
                                  ps[0:cnt].rearrange("p (h e) -> p h e", h=H))
            vsb[b][4 + it] = vt

    def attention_phase(b):
        for hp in range(4):
            aot = pool["ao"].tile([128, LQ], BF16, tag="ao", name="ao")
            pts = {0: [], 1: []}  # per head-in-pair exp'd S^T tiles
            for jt in range(7):
                cnt = JT_CNT[jt]
                if jt < 4:
                    ksrc, coff = kst[b][hp], jt * 128
                else:
                    ksrc, coff = qkT[b][4 + hp], (jt - 4) * 128
                for half in (0, 1):
                    ps = pool["ps_s"].tile([128, LQ], F32, tag="ps_s", name="ps_s")
                    nc.tensor.matmul(
                        ps[0:cnt, :],
                        ksrc[half * 64:half * 64 + 64, coff:coff + cnt],
                        qkT[b][hp][half * 64:half * 64 + 64, :],
                        start=True, stop=True,
                        tile_position=(half * 64, 0))
                    if jt == 6:
                        nc.vector.tensor_tensor(
                            out=ps[0:cnt, :], in0=ps[0:cnt, :],
                            in1=tabs["mk6"][0:cnt, :], op=ALU.add)
                    pt = pool["pt"].tile([128, LQ], BF16, tag="pt", name="pt")
                    nc.scalar.activation(pt[0:cnt, :], ps[0:cnt, :], AF.Exp)
                    pts[half].append(pt)
            for half in (0, 1):
                h = 2 * hp + half
                po = pool["ps_o"].tile([DH + 1, LQ], F32, tag="ps_o", name="ps_o")
                for jt in range(7):
                    cnt = JT_CNT[jt]
                    nc.tensor.matmul(po, vsb[b][jt][0:cnt, h, :],
                                     pts[half][jt][0:cnt, :],
                                     start=(jt == 0), stop=(jt == 6))
                rec = pool["rec"].tile([1, LQ], F32, tag="rec", name="rec")
                nc.vector.reciprocal(rec, po[64:65, :])
                pb = pool["ps_b"].tile([64, LQ], F32, tag="ps_b", name="ps_b")
                nc.tensor.matmul(pb, ones64, rec, start=True, stop=True)
                bc = pool["bc"].tile([64, LQ], F32, tag="bc", name="bc")
                nc.scalar.activation(bc, pb, AF.Copy)
                nc.vector.tensor_tensor(out=aot[half * 64:half * 64 + 64, :],
                                        in0=po[0:DH, :], in1=bc, op=ALU.mult)
            ao[b][hp] = aot

    def gate_out_phase(b):
        sgs = []
        for ot in range(4):
            ps = pool["ps_mm"].tile([128, NK], F32, tag="mm", name="mm")
            for fc in range(4):
                nc.tensor.matmul(ps[:, 0:LQ], wg[fc][:, ot * 128:(ot + 1) * 128],
                                 ao[b][fc], start=(fc == 0), stop=(fc == 3))
            s = pool["sg"].tile([128, LQ], BF16, tag="sg", name="sg")
            nc.scalar.activation(s, ps[:, 0:LQ], AF.Sigmoid)
            sgs.append(s)
        for ot in range(4):
            nc.vector.tensor_tensor(out=ao[b][ot], in0=ao[b][ot], in1=sgs[ot],
                                    op=ALU.mult)
        yn = pool["ynat"].tile([128, 3, D], BF16, tag="ynat", name="ynat")
        for ot in range(4):
            ps = pool["ps_mm"].tile([128, NK], F32, tag="mm", name="mm")
            for fc in range(4):
                nc.tensor.matmul(ps[:, 0:LQ], wo[fc][:, ot * 128:(ot + 1) * 128],
                                 ao[b][fc], start=(fc == 0), stop=(fc == 3))
            yb = pool["ybf"].tile([128, 384], BF16, tag="ybf", name="ybf")
            nc.gpsimd.memset(yb[:, LQ:384], 0.0)
            nc.scalar.activation(yb[:, 0:LQ], ps[:, 0:LQ], AF.Copy)
            nc.sync.dma_start(out=yn[:, :, ot * 128:(ot + 1) * 128],
                              in_=yb.rearrange("p (c q) -> p c q", c=3),
                              transpose=True)
        for it in range(3):
            cnt = IT_CNT[it]
            nc.vector.tensor_tensor(out=onat[b][it][0:cnt, :],
                                    in0=onat[b][it][0:cnt, :],
                                    in1=yn[0:cnt, it, :], op=ALU.add)

    def ffn_phase(b):
        n2T = pool["n2T"].tile([128, 4, 384], BF16, tag="n2T", name="n2T")
        for it in range(3):
            cnt = IT_CNT[it]
            r1 = onat[b][it]
            sqt = pool["sq"].tile([128, D], F32, tag="sq", name="sq")
            ss = pool["stats"].tile([128, 1], F32, tag="ss", name="ss")
            nc.scalar.activation(sqt[0:cnt, :], r1[0:cnt, :], AF.Square,
                                 accum_out=ss[0:cnt, :])
            std = pool["stats"].tile([128, 1], F32, tag="std", name="std")
            nc.scalar.activation(std[0:cnt, :], ss[0:cnt, :], AF.Sqrt,
                                 scale=1.0 / D, bias=epsb[0:cnt, :])
            rstd = pool["stats"].tile([128, 1], F32, tag="rstd", name="rstd")
            nc.vector.reciprocal(rstd[0:cnt, :], std[0:cnt, :])
            n2 = pool["n2bf"].tile([128, D], BF16, tag="xn", name="xn")
            if cnt < 128:
                nc.gpsimd.memset(n2[96:128, :], 0.0)
            nc.vector.tensor_scalar(out=n2[0:cnt, :], in0=r1[0:cnt, :],
                                    scalar1=rstd[0:cnt, :], scalar2=None,
                                    op0=ALU.mult)
            nc.sync.dma_start(out=n2T[:, :, it * 128:(it + 1) * 128],
                              in_=n2.rearrange("p (c q) -> p c q", c=4),
                              transpose=True)
        h1 = []
        for ot in range(4):
            ps = pool["ps_mm"].tile([128, NK], F32, tag="mm", name="mm")
            for fc in range(4):
                nc.tensor.matmul(ps[:, 0:LQ], w1[fc][:, ot * 128:(ot + 1) * 128],
                                 n2T[:, fc, 0:LQ], start=(fc == 0),
                                 stop=(fc == 3))
            ht = pool["h1T"].tile([128, LQ], BF16, tag="h1T", name="h1T")
            nc.scalar.activation(ht, ps[:, 0:LQ], AF.Gelu)
            h1.append(ht)
        f2n = pool["f2nat"].tile([128, 3, D], BF16, tag="f2nat", name="f2nat")
        for ot in range(4):
            ps = pool["ps_mm"].tile([128, NK], F32, tag="mm", name="mm")
            for fc in range(4):
                nc.tensor.matmul(ps[:, 0:LQ], w2[fc][:, ot * 128:(ot + 1) * 128],
                                 h1[fc], start=(fc == 0), stop=(fc == 3))
            fb = pool["f2bf"].tile([128, 384], BF16, tag="f2bf", name="f2bf")
            nc.gpsimd.memset(fb[:, LQ:384], 0.0)
            nc.scalar.activation(fb[:, 0:LQ], ps[:, 0:LQ], AF.Copy)
            nc.sync.dma_start(out=f2n[:, :, ot * 128:(ot + 1) * 128],
                              in_=fb.rearrange("p (c q) -> p c q", c=3),
                              transpose=True)
        for it in range(3):
            cnt = IT_CNT[it]
            nc.vector.tensor_tensor(out=onat[b][it][0:cnt, :],
                                    in0=onat[b][it][0:cnt, :],
                                    in1=f2n[0:cnt, it, :], op=ALU.add)
            nc.sync.dma_start(out=d["d_out"][b, it * 128:it * 128 + cnt, :],
                              in_=onat[b][it][0:cnt, :])

    # ---- program order (PE-readiness driven)
    import os
    lvl = int(os.environ.get('KERNEL_PHASES', '9'))
    xt0 = vlm_phase(0)
    state_phase(0, xt0)
    if lvl >= 2:
        norm1_phase(0)
        qkv_phase(0)
    if lvl >= 6:
        xt1 = vlm_phase(1)      # DMA/cast work overlaps b0 attention
    if lvl >= 3:
        attention_phase(0)
    if lvl >= 6:
        state_phase(1, xt1)
        norm1_phase(1)
        qkv_phase(1)
    if lvl >= 4:
        gate_out_phase(0)
    if lvl >= 6:
        attention_phase(1)
    if lvl >= 5:
        ffn_phase(0)
    if lvl >= 6:
        gate_out_phase(1)
        ffn_phase(1)
    ctx.close()


# ------------------------------------------------------------------- runner
_CACHED = {}


def _get_nc():
    if "nc" not in _CACHED:
        nc = build_nc()
        _split_oversized_waits(nc)
        _CACHED["nc"] = nc
    return _CACHED["nc"]


def prep_in_maps(vlm_keys, vlm_values, planning_tokens, t_tokens,
                 proprio_tokens, action_tokens, W_skv, b_skv, W_qkv, b_qkv,
                 W_gate, b_gate, W_out, b_out, W_ffn1, b_ffn1, W_ffn2,
                 b_ffn2, g_norm1, g_norm2):
    asnp = lambda x: np.asarray(x)
    vlm_keys, vlm_values = asnp(vlm_keys), asnp(vlm_values)
    orig = np.concatenate([asnp(planning_tokens), asnp(t_tokens),
                           asnp(proprio_tokens), asnp(action_tokens)],
                          axis=1).astype(np.float32)

    for bias in (b_skv, b_qkv, b_gate, b_out, b_ffn1, b_ffn2):
        assert np.max(np.abs(asnp(bias))) == 0.0, \
            "nonzero biases not supported by this kernel build"

    g1 = 1.0 + asnp(g_norm1).astype(np.float64)
    g2 = 1.0 + asnp(g_norm2).astype(np.float64)
    W_qkv = asnp(W_qkv).astype(np.float64)
    wqk = (g1[:, None] * W_qkv[:, :2 * D]).astype(BF16NP)
    wv = (g1[:, None] * W_qkv[:, 2 * D:]).astype(BF16NP)
    w1 = (g2[:, None] * asnp(W_ffn1).astype(np.float64)).astype(BF16NP)
    wskv = asnp(W_skv).astype(BF16NP)
    wg = asnp(W_gate).astype(BF16NP)
    wo = asnp(W_out).astype(BF16NP)
    w2 = asnp(W_ffn2).astype(BF16NP)

    cq, sq = _rope_tables(np.arange(LQ), SCALE)
    cks, sks = _rope_tables(np.arange(NK, LK), 1.0)
    ckst, skst = _rope_tables(np.arange(NK), 1.0)
    mk6 = _mask6_table()

    shared = dict(wskv=wskv, wqk=wqk, wv=wv, wg=wg, wo=wo, w1=w1, w2=w2,
                  cq=cq, sq=sq, cks=cks, sks=sks, ckst=ckst, skst=skst,
                  mk6=mk6)
    in_maps = []
    for cid in range(CORES):
        sl = slice(cid * BL, (cid + 1) * BL)
        in_maps.append(dict(
            vk=np.ascontiguousarray(vlm_keys[sl], np.float32),
            vv=np.ascontiguousarray(vlm_values[sl], np.float32),
            orig=np.ascontiguousarray(orig[sl]),
            **shared))
    return in_maps


def _split_outs(out):
    return (out[:, :P], out[:, P:P + 1], out[:, P + 1:P + 1 + PR],
            out[:, P + 1 + PR:])


def kernel(**inputs):
    from concourse.bass_utils import run_bass_kernel_spmd
    in_maps = prep_in_maps(**inputs)
    nc = _get_nc()
    res = run_bass_kernel_spmd(nc, in_maps, list(range(CORES)))
    out = np.concatenate([res.results[cid]["out"] for cid in range(CORES)],
                         axis=0).astype(np.float32)
    return _split_outs(out)
